# revision 7
# baseline (speedup 1.0000x reference)
"""GRU decoder (teacher forcing) + log_softmax on 8 Trainium2 NeuronCores.

Strategy (v3):
  - Vocab-shard the projection/log-softmax across the 8 cores (W_proj rows),
    replicate the (tiny, serial) GRU recurrence on every core.
  - Phase 0 (per 8-step chunk): indirect-DMA gather of embedding rows,
    DMA-xbar transpose to k-major, matmul -> x-side gate pre-activations
    XG = emb @ W_ih.T (+ b_ih + b_hh[r,z]) stored time-major in SBUF.
  - Phase 1 (63 sequential steps): hg^T = W_hh^T-slabs.T @ h^T on PE
    (weights stationary, batch on the moving free axis); all 12 gate chunks
    share one PSUM bank; tanh(r) fires as soon as the r chunks are done so
    the n-gate chain starts early; z is finished while n is in flight.
    sigmoid(x) = 0.5*tanh(x/2)+0.5 so only the exp_and_others ACT table is
    ever loaded (tanh+exp+identity live there; no table switches).
  - Phase 2 (16 row-tiles of 128): logits = HT-slabs.T @ W_projT-shard with
    fp8e4 DoubleRow matmuls (W_proj pre-scaled x64 on host, h x8 on device;
    PSUM holds 512*logit in [128,2,512] double-unit tiles).  One exp
    (scale=1/512, bias=-4ln2) per 1000 cols with accum_out row partial
    sums; true logits parked in fp16 SBUF rings via one DVE scale per
    1000 cols.  Small AllGathers exchange partial sums per group of
    row-tiles; group 0 is a single row-tile so the first (slow, path-
    warming) collective fires early with its readers delayed far enough
    to never block an engine queue.  lse via DVE bit-twiddle log; final
    out = logit - lse on DVE (4x fp16 tensor_scalar, per-row AP scalar),
    DMA'd as fp16 and upcast on host.

kernel(**inputs) takes the FULL numpy inputs, does layout prep on host,
runs the SPMD NEFF on cores 0..7 and reassembles the [32, 64, 32000] output.
"""

import os

import numpy as np
import ml_dtypes

import concourse.bass as bass
import concourse.bacc as bacc
import concourse.mybir as mybir
import concourse.tile as tile
from concourse.bass_utils import run_bass_kernel_spmd
from concourse.masks import make_identity

# problem shape (hardcoded per contract)
B, T, V, E, H = 32, 64, 32000, 256, 512
S = T - 1                 # 63 decode steps
NCORES = 8
VS = V // NCORES          # 4000 vocab shard per core
G = 3 * H                 # 1536 gate dims
GC = G // 128             # 12 gate chunks
KH = H // 128             # 4 contraction tiles over H
KE = E // 128             # 2 contraction tiles over E
NROW = S * B              # 2016 output rows, (t, b) order
NMT = (NROW + 127) // 128  # 16 row-tiles (last has 96 rows)
# stat-collective groups (start_mtile, count) and reader release delays
GROUPS = [(0, 1), (1, 3), (4, 4), (8, 4), (12, 3), (15, 1)]
GDELAY = [40, 10, 10, 10, 8, 2]
GRP_OF_M = {}
for _gi, (_s, _c) in enumerate(GROUPS):
    for _m in range(_s, _s + _c):
        GRP_OF_M[_m] = _gi
NV2 = VS // 1000          # 4 double-units (1000 vocab) per row-tile
LN2 = float(np.log(2.0))
EXP_BIAS = -4.0 * LN2     # exp(logit - 4ln2): keeps fp16 exp safely in range
WSCL = 64.0               # host pre-scale of W_proj before fp8 cast
HSCL = 8.0                # device pre-scale of h before fp8 cast
PSCL = WSCL * HSCL        # PSUM = PSCL * logit

F32 = mybir.dt.float32
BF16 = mybir.dt.bfloat16
F16 = mybir.dt.float16
F8 = mybir.dt.float8e4
I32 = mybir.dt.int32
U32 = mybir.dt.uint32
AF = mybir.ActivationFunctionType
OP = mybir.AluOpType
DRow = mybir.MatmulPerfMode.DoubleRow

# -ln(m) Chebyshev-interpolation coefficients on m in [1, 2], highest first.
_nodes = np.cos((2 * np.arange(1, 7) - 1) / (2 * 6.0) * np.pi) * 0.5 + 1.5
_NEGLN_COEF = [float(c) for c in np.polyfit(_nodes, -np.log(_nodes), 5)]

_BUILD_CACHE = {}


def _build(bhh_n_nonzero: bool, bproj_nonzero: bool, bx_nonzero: bool):
    debug = bool(int(os.environ.get("KERNEL_DEBUG", "0")))
    key = (bhh_n_nonzero, bproj_nonzero, bx_nonzero, debug)
    if key in _BUILD_CACHE:
        return _BUILD_CACHE[key]

    nc = bacc.Bacc("TRN2", target_bir_lowering=False, debug=False,
                   enable_asserts=False, num_devices=NCORES)

    trg_d = nc.dram_tensor("trg_flat", (NROW, 1), I32, kind="ExternalInput")
    tbl_d = nc.dram_tensor("emb_tbl", (V, E), BF16, kind="ExternalInput")
    wih_d = nc.dram_tensor("wih_t", (128, KE, G), BF16, kind="ExternalInput")
    whh_d = nc.dram_tensor("whh_t", (128, KH, G), BF16, kind="ExternalInput")
    h0_d = nc.dram_tensor("h0_t", (128, KH, B), BF16, kind="ExternalInput")
    wpr_d = nc.dram_tensor("wproj_t", (128, KH, VS), F8, kind="ExternalInput")
    bx_d = nc.dram_tensor("bx_t", (128, GC), BF16, kind="ExternalInput")
    if bhh_n_nonzero:
        bhn_d = nc.dram_tensor("bhn_t", (128, KH), BF16, kind="ExternalInput")
    if bproj_nonzero:
        bpr_d = nc.dram_tensor("bproj_s", (1, VS), F32, kind="ExternalInput")
    out_d = nc.dram_tensor("out_lp", (NROW, VS), F16, kind="ExternalOutput")
    if debug:
        ht_d = nc.dram_tensor("dbg_ht", (128, KH, NROW), BF16,
                              kind="ExternalOutput")
        xg_d = nc.dram_tensor("dbg_xg", (128, 8, GC, B), BF16,
                              kind="ExternalOutput")
        sall_d = nc.dram_tensor("dbg_sall", (128, NMT * NV2), F32,
                                kind="ExternalOutput")
        lg_d = nc.dram_tensor("dbg_lg", (128, VS), F16, kind="ExternalOutput")
        nlse_d = nc.dram_tensor("dbg_nlse", (128, 1), F32,
                                kind="ExternalOutput")

    with tile.TileContext(nc) as tc:
        with tc.tile_pool(name="sb", bufs=1) as sb, \
             tc.tile_pool(name="ps", bufs=1, space="PSUM") as ps, \
             tc.tile_pool(name="dram", bufs=1, space="DRAM") as dp:

            # ---------------- phase 0 helpers -------------------------------
            xg_tiles = {}

            def emit_prep_gather(c8):
                tlo = 8 * c8
                nst = min(8, S - tlo)
                nrows = B * nst
                xg = sb.tile([128, 8, GC, B], BF16, tag="xg", bufs=2,
                             name=f"xg{c8}")
                xg_tiles[c8] = xg
                embt = sb.tile([128, KE, 256], BF16, tag="embt", bufs=2,
                               name=f"embt{c8}")
                for sub in range(2):
                    lo = tlo * B + sub * 128
                    nr = min(128, nrows - sub * 128)
                    if nr <= 0:
                        continue
                    idx_t = sb.tile([128, 1], I32, tag="idx", bufs=4,
                                    name=f"idx{c8}_{sub}")
                    nc.sync.dma_start(idx_t[:nr], trg_d[lo:lo + nr, :])
                    rows = sb.tile([128, E], BF16, tag="embr", bufs=4,
                                   name=f"embr{c8}_{sub}")
                    nc.gpsimd.indirect_dma_start(
                        out=rows[:nr], out_offset=None, in_=tbl_d[:],
                        in_offset=bass.IndirectOffsetOnAxis(ap=idx_t[:nr, :1], axis=0))
                    for kb in range(KE):
                        nc.sync.dma_start_transpose(
                            embt[:, kb, sub * 128:sub * 128 + nr],
                            rows[:nr, kb * 128:(kb + 1) * 128])
                return embt

            def emit_prep_xg(c8, embt, gps):
                # gps: list of even gate-chunk starts; processes pairs
                tlo = 8 * c8
                nst = min(8, S - tlo)
                nrows = B * nst
                xg = xg_tiles[c8]
                for gp in gps:
                    pxg = ps.tile([128, 2, 256], F32, tag="ps_xg", bufs=1,
                                  name=f"pxg{c8}_{gp}")
                    for gi in range(2):
                        gc = gp + gi
                        for kt in range(KE):
                            nc.tensor.matmul(
                                pxg[:, gi, :nrows],
                                lhsT=wih_sb[:, kt, gc * 128:(gc + 1) * 128],
                                rhs=embt[:, kt, :nrows],
                                start=(kt == 0), stop=(kt == KE - 1))
                    src = pxg[:, :, :nrows].rearrange(
                        "p g (t b) -> p t g b", b=B)
                    if bx_nonzero:
                        nc.vector.tensor_tensor(
                            out=xg[:, :nst, gp:gp + 2, :], in0=src,
                            in1=bx_sb[:, None, gp:gp + 2, None].to_broadcast(
                                [128, nst, 2, B]),
                            op=OP.add)
                    else:
                        nc.vector.tensor_copy(xg[:, :nst, gp:gp + 2, :], src)

            # ------- startup: the xg(0) chain first (it's the longest pole),
            # persistent loads next (they overlap the gather/transpose chain).
            embt0 = emit_prep_gather(0)
            h0_sb = sb.tile([128, KH, B], BF16)
            nc.scalar.dma_start(h0_sb[:], h0_d[:])
            whh_sb = sb.tile([128, KH, G], BF16)
            nc.scalar.dma_start(whh_sb[:], whh_d[:])
            wih_sb = sb.tile([128, KE, G], BF16)
            nc.scalar.dma_start(wih_sb[:], wih_d[:])
            bx_sb = sb.tile([128, GC], BF16)
            nc.scalar.dma_start(bx_sb[:], bx_d[:])
            if bhh_n_nonzero:
                bhn_sb = sb.tile([128, KH], BF16)
                nc.scalar.dma_start(bhn_sb[:], bhn_d[:])

            HT = sb.tile([128, KH, NROW], BF16)      # h_{t+1} states, (t, b) cols
            ebias = sb.tile([128, 1], F32)
            nc.gpsimd.memset(ebias[:], EXP_BIAS)
            S_all = sb.tile([128, NMT * NV2], F32)   # exp partial sums
            nc.gpsimd.memset(S_all[:], 0.0)
            ident = sb.tile([128, 128], BF16)
            make_identity(nc, ident[:])

            emit_prep_xg(0, embt0, range(0, GC, 2))
            if debug:
                nc.sync.dma_start(xg_d[:], xg_tiles[0][:])

            # W_proj shard: large, first needed by phase 2 -> load after the
            # startup-critical tensors so it doesn't congest the DMA queues.
            wpr_sb = sb.tile([128, KH, VS], F8)
            nc.sync.dma_start(wpr_sb[:], wpr_d[:])
            if bproj_nonzero:
                bpr_sb = sb.tile([128, VS], F32)
                nc.gpsimd.dma_start(bpr_sb[:], bpr_d[:1, :].to_broadcast([128, VS]))

            # ---------------- phase 1 step ----------------------------------
            def emit_step(t):
                h_prev = h0_sb[:, :, :] if t == 0 else HT[:, :, (t - 1) * B:t * B]
                xg = xg_tiles[t // 8][:, t % 8, :, :]
                # gate chunk layout in one PSUM bank: r 0-3, z 4-7, n 8-11
                pa = ps.tile([128, 12, B], F32, tag="ps_a", name=f"pa{t}")
                # r first (the n-chain needs it), then n, then z
                for gc in range(4):
                    for kt in range(KH):
                        nc.tensor.matmul(
                            pa[:, gc, :],
                            lhsT=whh_sb[:, kt, gc * 128:(gc + 1) * 128],
                            rhs=h_prev[:, kt, :],
                            start=(kt == 0), stop=False)
                for gc in range(4):
                    nc.tensor.matmul(
                        pa[:, gc, :], lhsT=ident[:], rhs=xg[:, gc, :],
                        start=False, stop=True)
                for gc in range(8, 12):
                    for kt in range(KH):
                        nc.tensor.matmul(
                            pa[:, gc, :],
                            lhsT=whh_sb[:, kt, gc * 128:(gc + 1) * 128],
                            rhs=h_prev[:, kt, :],
                            start=(kt == 0), stop=(kt == KH - 1))
                for gc in range(4, 8):
                    for kt in range(KH):
                        nc.tensor.matmul(
                            pa[:, gc, :],
                            lhsT=whh_sb[:, kt, gc * 128:(gc + 1) * 128],
                            rhs=h_prev[:, kt, :],
                            start=(kt == 0), stop=False)
                for gc in range(4, 8):
                    nc.tensor.matmul(
                        pa[:, gc, :], lhsT=ident[:], rhs=xg[:, gc, :],
                        start=False, stop=True)
                # r gate: sigma(x) = 0.5*tanh(x/2) + 0.5
                rt = sb.tile([128, 4, B], BF16, tag="rt", bufs=2, name=f"rt{t}")
                nc.scalar.activation(rt[:], pa[:, 0:4, :], AF.Tanh, scale=0.5)
                r_s = sb.tile([128, 4, B], BF16, tag="r_s", bufs=2, name=f"rs{t}")
                nc.vector.tensor_scalar(out=r_s[:], in0=rt[:], scalar1=0.5,
                                        scalar2=0.5, op0=OP.mult, op1=OP.add)
                # n gate
                if bhh_n_nonzero:
                    nc.vector.tensor_tensor(
                        out=pa[:, 8:12, :], in0=pa[:, 8:12, :],
                        in1=bhn_sb[:, :, None].to_broadcast([128, 4, B]), op=OP.add)
                nc.vector.tensor_tensor(out=pa[:, 8:12, :], in0=pa[:, 8:12, :],
                                        in1=r_s[:], op=OP.mult)
                nc.vector.tensor_tensor(out=pa[:, 8:12, :], in0=pa[:, 8:12, :],
                                        in1=xg[:, 8:12, :], op=OP.add)
                n_s = sb.tile([128, 4, B], BF16, tag="n_s", bufs=2, name=f"ns{t}")
                nc.scalar.activation(n_s[:], pa[:, 8:12, :], AF.Tanh)
                # z gate (off the critical path until the final blend)
                zt = sb.tile([128, 4, B], BF16, tag="zt", bufs=2, name=f"zt{t}")
                nc.scalar.activation(zt[:], pa[:, 4:8, :], AF.Tanh, scale=0.5)
                z_s = sb.tile([128, 4, B], BF16, tag="z_s", bufs=2, name=f"zs{t}")
                nc.gpsimd.tensor_scalar(out=z_s[:], in0=zt[:], scalar1=0.5,
                                        scalar2=0.5, op0=OP.mult, op1=OP.add)
                q_s = sb.tile([128, 4, B], BF16, tag="q_s", bufs=2, name=f"qs{t}")
                nc.gpsimd.tensor_scalar(out=q_s[:], in0=zt[:], scalar1=-0.5,
                                        scalar2=0.5, op0=OP.mult, op1=OP.add)
                p_s = sb.tile([128, 4, B], BF16, tag="p_s", bufs=2, name=f"ps{t}")
                nc.gpsimd.tensor_tensor(out=p_s[:], in0=z_s[:], in1=h_prev,
                                        op=OP.mult)
                # h' = n*(1-z) + z*h
                w_s = sb.tile([128, 4, B], BF16, tag="w_s", bufs=2, name=f"ws{t}")
                nc.vector.tensor_tensor(out=w_s[:], in0=n_s[:], in1=q_s[:],
                                        op=OP.mult)
                nc.vector.tensor_tensor(out=HT[:, :, t * B:(t + 1) * B],
                                        in0=w_s[:], in1=p_s[:], op=OP.add)

            # ---------------- phase 2 emission helpers ----------------------
            logit_tiles = {}
            ht8_tiles = {}
            lse_tiles = {}

            def emit_munit(m, u2):
                # one 1000-vocab double-unit of row-tile m's logits + stats
                mp = min(128, NROW - m * 128)
                if u2 == 0:
                    logit_tiles[m] = sb.tile([128, VS], F16, tag="logit",
                                             bufs=9, name=f"lg{m}")
                    ht8 = sb.tile([128, KH, 128], F8, tag="ht8", bufs=2,
                                  name=f"ht8_{m}")
                    ht8_tiles[m] = ht8
                    nc.vector.tensor_scalar(
                        out=ht8[:, :, :mp], in0=HT[:, :, m * 128:m * 128 + mp],
                        scalar1=HSCL, scalar2=None, op0=OP.mult)
                lg = logit_tiles[m]
                ht8 = ht8_tiles[m]
                pl = ps.tile([128, 2, 512], F32, tag="ps_l", bufs=3,
                             name=f"pl{m}_{u2}")
                for half in range(2):
                    for kp in range(KH // 2):
                        nc.tensor.matmul(
                            pl[:mp, half, :500],
                            lhsT=ht8[:, 2 * kp:2 * kp + 2, :mp],
                            rhs=wpr_sb[:, 2 * kp:2 * kp + 2,
                                       u2 * 1000 + half * 500:
                                       u2 * 1000 + half * 500 + 500],
                            start=(kp == 0), stop=(kp == KH // 2 - 1),
                            perf_mode=DRow)
                src = pl[:mp, :, :500]
                if bproj_nonzero:
                    # bproj_s is pre-scaled by PSCL on the host
                    nc.vector.tensor_tensor(
                        out=src, in0=src,
                        in1=bpr_sb[:mp, u2 * 1000:(u2 + 1) * 1000].rearrange(
                            "p (a b) -> p a b", a=2), op=OP.add)
                # true logits (fp16) parked until the group's lse is known
                nc.vector.tensor_scalar(
                    out=lg[:mp, u2 * 1000:(u2 + 1) * 1000].rearrange(
                        "p (a b) -> p a b", a=2),
                    in0=src, scalar1=1.0 / PSCL, scalar2=None, op0=OP.mult)
                esc = sb.tile([128, 2, 500], F16, tag="exps", bufs=2,
                              name=f"esc{m}_{u2}")
                nc.scalar.activation(
                    esc[:mp], src, AF.Exp, bias=ebias[:mp, :1],
                    scale=1.0 / PSCL,
                    accum_out=S_all[:mp, m * NV2 + u2:m * NV2 + u2 + 1])

            def emit_group_trigger(g):
                # local row-sums + AllGather trigger; readers emitted later
                mlo, cnt = GROUPS[g]
                sg = sb.tile([128, 4], F32, tag="sg", bufs=2, name=f"sg{g}")
                for j in range(cnt):
                    m = mlo + j
                    nc.vector.reduce_sum(
                        out=sg[:, j:j + 1],
                        in_=S_all[:, m * NV2:(m + 1) * NV2],
                        axis=mybir.AxisListType.X)
                cin = dp.tile([128, cnt], F32, tag=f"cin{g}", name=f"cin{g}")
                nc.gpsimd.dma_start(cin[:], sg[:, :cnt])
                cout = dp.tile([NCORES, 128, cnt], F32, tag=f"cout{g}",
                               addr_space="Shared", name=f"cout{g}")
                nc.gpsimd.collective_compute(
                    "AllGather", OP.bypass,
                    replica_groups=[list(range(NCORES))],
                    ins=[cin.opt()], outs=[cout.opt()])
                return cout

            def emit_group_stats(g, cout):
                mlo, cnt = GROUPS[g]
                s8 = sb.tile([128, cnt, NCORES], F32, tag="s8", bufs=2,
                             name=f"s8{g}")
                nc.gpsimd.dma_start(s8[:], cout[:].rearrange("c p m -> p m c"))
                st = sb.tile([128, cnt], F32, tag="st", bufs=2, name=f"st{g}")
                nc.vector.reduce_sum(out=st[:], in_=s8[:],
                                     axis=mybir.AxisListType.X)
                # neg_lse = -(e - 127 + 4) * ln2 - ln(m),  St = m * 2^(e-127)
                iu = st[:].bitcast(U32)
                eu = sb.tile([128, cnt], U32, tag="eu", bufs=2, name=f"eu{g}")
                nc.vector.tensor_scalar(out=eu[:], in0=iu, scalar1=23,
                                        scalar2=None, op0=OP.logical_shift_right)
                ef = sb.tile([128, cnt], F32, tag="ef", bufs=2, name=f"ef{g}")
                nc.vector.tensor_copy(ef[:], eu[:])
                mu = sb.tile([128, cnt], U32, tag="mu", bufs=2, name=f"mu{g}")
                nc.vector.tensor_scalar(out=mu[:], in0=iu, scalar1=0x007FFFFF,
                                        scalar2=0x3F800000, op0=OP.bitwise_and,
                                        op1=OP.bitwise_or)
                mf = mu[:].bitcast(F32)
                acc = sb.tile([128, cnt], F32, tag="acc", bufs=2, name=f"acc{g}")
                c = _NEGLN_COEF
                nc.vector.tensor_scalar(out=acc[:], in0=mf, scalar1=c[0],
                                        scalar2=c[1], op0=OP.mult, op1=OP.add)
                for k in range(2, 6):
                    nc.vector.tensor_tensor(out=acc[:], in0=acc[:], in1=mf,
                                            op=OP.mult)
                    nc.vector.tensor_scalar(out=acc[:], in0=acc[:], scalar1=c[k],
                                            scalar2=None, op0=OP.add)
                # + (127 - 4 - e) * ln2   (the -4 re-adds the exp bias so
                #   lse refers to unshifted logits)
                e2 = sb.tile([128, cnt], F32, tag="e2", bufs=2, name=f"e2{g}")
                nc.vector.tensor_scalar(out=e2[:], in0=ef[:], scalar1=-LN2,
                                        scalar2=(127.0 - 4.0) * LN2,
                                        op0=OP.mult, op1=OP.add)
                nlse = sb.tile([128, cnt], F32, tag="nlse", bufs=2,
                               name=f"nlse{g}")
                nc.vector.tensor_tensor(out=nlse[:], in0=acc[:], in1=e2[:],
                                        op=OP.add)
                lse_tiles[g] = nlse
                if debug and g == 0:
                    nc.sync.dma_start(nlse_d[:], nlse[:])

            def emit_output(m):
                g = GRP_OF_M[m]
                j = m - GROUPS[g][0]
                mp = min(128, NROW - m * 128)
                nlse = lse_tiles[g]
                lg = logit_tiles.pop(m)
                ht8_tiles.pop(m, None)
                if debug and m == 0:
                    nc.sync.dma_start(lg_d[:], lg[:])
                ot = sb.tile([128, VS], F16, tag="ot", bufs=2, name=f"ot{m}")
                nc.vector.tensor_scalar(out=ot[:mp], in0=lg[:mp],
                                        scalar1=nlse[:mp, j:j + 1],
                                        scalar2=None, op0=OP.add)
                nc.sync.dma_start(out_d[m * 128:m * 128 + mp, :], ot[:mp])

            # ---------------- main emission loop ----------------------------
            # Interleave prep / phase-2 work between steps in small pieces so
            # the scheduler can't starve the serial recurrence on PE.  Items
            # that read an AllGather result are released several drain slots
            # after the trigger so no engine queue blocks on AG latency; the
            # per-group outputs release one per slot to avoid bursts.
            from collections import deque
            work_q = deque()
            delayed = []          # (release_tick, fn), kept sorted
            tick = 0

            def fin_ready(g):
                def trig(g=g):
                    cout = emit_group_trigger(g)
                    rel = tick + GDELAY[g]
                    delayed.append((rel, lambda g=g, cout=cout:
                                    emit_group_stats(g, cout)))
                    mlo, cnt = GROUPS[g]
                    for k in range(cnt):
                        delayed.append((rel + 1 + k,
                                        lambda m=mlo + k: emit_output(m)))
                    delayed.sort(key=lambda x: x[0])
                return trig

            def enqueue_mtile(m):
                for u2 in range(NV2):
                    work_q.append(lambda m=m, u2=u2: emit_munit(m, u2))
                for g, (mlo, cnt) in enumerate(GROUPS):
                    if m == mlo + cnt - 1:
                        work_q.append(fin_ready(g))

            def drain(n):
                nonlocal tick
                for _ in range(n):
                    tick += 1
                    if delayed and delayed[0][0] <= tick:
                        delayed.pop(0)[1]()
                    elif work_q:
                        work_q.popleft()()
                    else:
                        break

            for t in range(S):
                emit_step(t)
                if t % 8 == 1 and t // 8 + 1 <= (S - 1) // 8:
                    c8 = t // 8 + 1
                    embt = emit_prep_gather(c8)
                    for lo in range(0, GC, 4):
                        work_q.append(lambda c8=c8, embt=embt, lo=lo:
                                      emit_prep_xg(c8, embt, range(lo, lo + 4, 2)))
                if t >= 3 and (t - 3) % 4 == 0:
                    enqueue_mtile((t - 3) // 4)
                drain(3)
            for m in range(((S - 1 - 3) // 4) + 1, NMT):
                enqueue_mtile(m)
            while work_q or delayed:
                if not work_q and delayed:
                    tick = max(tick, delayed[0][0] - 1)
                drain(1)
            if debug:
                nc.sync.dma_start(ht_d[:], HT[:])
                nc.sync.dma_start(sall_d[:], S_all[:])

    nc.finalize()
    _BUILD_CACHE[key] = nc
    return nc


def _pack_T(w, ktiles, dtype=ml_dtypes.bfloat16, scale=1.0):
    """[out_dim, in_dim] f32 -> [128, ktiles, out_dim] (w.T, k-major slabs)."""
    wT = np.ascontiguousarray(w.T * scale).astype(dtype)
    return np.ascontiguousarray(
        wT.reshape(ktiles, 128, w.shape[0]).transpose(1, 0, 2))


LAST_PROFILE = None


def kernel(trg, h0, embed_table, W_ih, W_hh, b_ih, b_hh, W_proj, b_proj):
    global LAST_PROFILE
    trg = np.asarray(trg)
    h0 = np.asarray(h0, dtype=np.float32)
    embed_table = np.asarray(embed_table, dtype=np.float32)
    W_ih = np.asarray(W_ih, dtype=np.float32)
    W_hh = np.asarray(W_hh, dtype=np.float32)
    b_ih = np.asarray(b_ih, dtype=np.float32)
    b_hh = np.asarray(b_hh, dtype=np.float32)
    W_proj = np.asarray(W_proj, dtype=np.float32)
    b_proj = np.asarray(b_proj, dtype=np.float32)

    # bx = b_ih + [b_hh for r,z chunks; 0 for n chunks], packed [128, GC]
    bx = b_ih.copy()
    bx[:2 * H] += b_hh[:2 * H]
    bhh_n_nonzero = bool(np.any(b_hh[2 * H:]))
    bproj_nonzero = bool(np.any(b_proj))
    bx_nonzero = bool(np.any(bx))
    nc = _build(bhh_n_nonzero, bproj_nonzero, bx_nonzero)

    # host-side layout prep (sharding/packing only)
    trg_flat = np.ascontiguousarray(
        trg[:, :S].T.reshape(NROW, 1)).astype(np.int32)
    tbl_bf = embed_table.astype(ml_dtypes.bfloat16)
    wih_t = _pack_T(W_ih, KE)
    whh_t = _pack_T(W_hh, KH)
    h0_t = np.ascontiguousarray(
        h0[0].T.reshape(KH, 128, B).transpose(1, 0, 2)).astype(ml_dtypes.bfloat16)
    bx_t = np.ascontiguousarray(bx.reshape(GC, 128).T).astype(ml_dtypes.bfloat16)

    base = {
        "trg_flat": trg_flat,
        "emb_tbl": tbl_bf,
        "wih_t": wih_t,
        "whh_t": whh_t,
        "h0_t": h0_t,
        "bx_t": bx_t,
    }
    if bhh_n_nonzero:
        base["bhn_t"] = np.ascontiguousarray(
            b_hh[2 * H:].reshape(KH, 128).T).astype(ml_dtypes.bfloat16)

    in_maps = []
    for c in range(NCORES):
        m = dict(base)
        m["wproj_t"] = _pack_T(W_proj[c * VS:(c + 1) * VS], KH,
                               dtype=ml_dtypes.float8_e4m3, scale=WSCL)
        if bproj_nonzero:
            m["bproj_s"] = np.ascontiguousarray(
                (b_proj[c * VS:(c + 1) * VS] * PSCL).reshape(1, VS))
        in_maps.append(m)

    trace = bool(int(os.environ.get("KERNEL_TRACE", "0")))
    res = run_bass_kernel_spmd(nc, in_maps, core_ids=list(range(NCORES)),
                               trace=trace)
    LAST_PROFILE = res

    out = np.zeros((B, T, V), dtype=np.float32)
    big = np.stack([res.results[c]["out_lp"].reshape(S, B, VS)
                    for c in range(NCORES)], axis=0)   # [c, t, b, vs]
    out[:, 1:, :] = big.transpose(2, 1, 0, 3).reshape(B, S, V)
    return out


# revision 8
# speedup vs baseline: 1.1217x; 1.1217x over previous
"""GRU decoder (teacher forcing) + log_softmax on 8 Trainium2 NeuronCores.

Strategy (v3):
  - Vocab-shard the projection/log-softmax across the 8 cores (W_proj rows),
    replicate the (tiny, serial) GRU recurrence on every core.
  - Phase 0 (per 8-step chunk): indirect-DMA gather of embedding rows,
    DMA-xbar transpose to k-major, matmul -> x-side gate pre-activations
    XG = emb @ W_ih.T (+ b_ih + b_hh[r,z]) stored time-major in SBUF.
  - Phase 1 (63 sequential steps): hg^T = W_hh^T-slabs.T @ h^T on PE
    (weights stationary, batch on the moving free axis); all 12 gate chunks
    share one PSUM bank; tanh(r) fires as soon as the r chunks are done so
    the n-gate chain starts early; z is finished while n is in flight.
    sigmoid(x) = 0.5*tanh(x/2)+0.5 so only the exp_and_others ACT table is
    ever loaded (tanh+exp+identity live there; no table switches).
  - Phase 2 (16 row-tiles of 128): logits = HT-slabs.T @ W_projT-shard with
    fp8e4 DoubleRow matmuls (W_proj pre-scaled x64 on host, h x8 on device;
    PSUM holds 512*logit in [128,2,512] double-unit tiles).  One exp
    (scale=1/512, bias=-4ln2) per 1000 cols with accum_out row partial
    sums; true logits parked in fp16 SBUF rings via one DVE scale per
    1000 cols.  Small AllGathers exchange partial sums per group of
    row-tiles; group 0 is a single row-tile so the first (slow, path-
    warming) collective fires early with its readers delayed far enough
    to never block an engine queue.  lse via DVE bit-twiddle log; final
    out = logit - lse on DVE (4x fp16 tensor_scalar, per-row AP scalar),
    DMA'd as fp16 and upcast on host.

kernel(**inputs) takes the FULL numpy inputs, does layout prep on host,
runs the SPMD NEFF on cores 0..7 and reassembles the [32, 64, 32000] output.
"""

import os

import numpy as np
import ml_dtypes

import concourse.bass as bass
import concourse.bacc as bacc
import concourse.mybir as mybir
import concourse.tile as tile
from concourse.bass_utils import run_bass_kernel_spmd
from concourse.masks import make_identity

# problem shape (hardcoded per contract)
B, T, V, E, H = 32, 64, 32000, 256, 512
S = T - 1                 # 63 decode steps
NCORES = 8
VS = V // NCORES          # 4000 vocab shard per core
G = 3 * H                 # 1536 gate dims
GC = G // 128             # 12 gate chunks
KH = H // 128             # 4 contraction tiles over H
KE = E // 128             # 2 contraction tiles over E
NROW = S * B              # 2016 output rows, (t, b) order
NMT = (NROW + 127) // 128  # 16 row-tiles (last has 96 rows)
# stat-collective groups (start_mtile, count) and reader release delays
GROUPS = [(0, 1), (1, 3), (4, 4), (8, 4), (12, 3), (15, 1)]
GDELAY = [45, 30, 18, 12, 9, 3]
GRP_OF_M = {}
for _gi, (_s, _c) in enumerate(GROUPS):
    for _m in range(_s, _s + _c):
        GRP_OF_M[_m] = _gi
NV2 = VS // 1000          # 4 double-units (1000 vocab) per row-tile
LN2 = float(np.log(2.0))
EXP_BIAS = -4.0 * LN2     # exp(logit - 4ln2): keeps fp16 exp safely in range
WSCL = 64.0               # host pre-scale of W_proj before fp8 cast
HSCL = 8.0                # device pre-scale of h before fp8 cast
PSCL = WSCL * HSCL        # PSUM = PSCL * logit

F32 = mybir.dt.float32
BF16 = mybir.dt.bfloat16
F16 = mybir.dt.float16
F8 = mybir.dt.float8e4
I32 = mybir.dt.int32
U32 = mybir.dt.uint32
AF = mybir.ActivationFunctionType
OP = mybir.AluOpType
DRow = mybir.MatmulPerfMode.DoubleRow

# -ln(m) Chebyshev-interpolation coefficients on m in [1, 2], highest first.
_nodes = np.cos((2 * np.arange(1, 7) - 1) / (2 * 6.0) * np.pi) * 0.5 + 1.5
_NEGLN_COEF = [float(c) for c in np.polyfit(_nodes, -np.log(_nodes), 5)]

_BUILD_CACHE = {}


def _build(bhh_n_nonzero: bool, bproj_nonzero: bool, bx_nonzero: bool):
    debug = bool(int(os.environ.get("KERNEL_DEBUG", "0")))
    key = (bhh_n_nonzero, bproj_nonzero, bx_nonzero, debug)
    if key in _BUILD_CACHE:
        return _BUILD_CACHE[key]

    nc = bacc.Bacc("TRN2", target_bir_lowering=False, debug=False,
                   enable_asserts=False, num_devices=NCORES)

    trg_d = nc.dram_tensor("trg_flat", (NROW, 1), I32, kind="ExternalInput")
    tbl_d = nc.dram_tensor("emb_tbl", (V, E), BF16, kind="ExternalInput")
    wih_d = nc.dram_tensor("wih_t", (128, KE, G), BF16, kind="ExternalInput")
    whh_d = nc.dram_tensor("whh_t", (128, KH, G), BF16, kind="ExternalInput")
    h0_d = nc.dram_tensor("h0_t", (128, KH, B), BF16, kind="ExternalInput")
    wpr_d = nc.dram_tensor("wproj_t", (128, KH, VS), F8, kind="ExternalInput")
    bx_d = nc.dram_tensor("bx_t", (128, GC), BF16, kind="ExternalInput")
    if bhh_n_nonzero:
        bhn_d = nc.dram_tensor("bhn_t", (128, KH), BF16, kind="ExternalInput")
    if bproj_nonzero:
        bpr_d = nc.dram_tensor("bproj_s", (1, VS), F32, kind="ExternalInput")
    out_d = nc.dram_tensor("out_lp", (NROW, VS), F16, kind="ExternalOutput")
    if debug:
        ht_d = nc.dram_tensor("dbg_ht", (128, KH, NROW), BF16,
                              kind="ExternalOutput")
        xg_d = nc.dram_tensor("dbg_xg", (128, 8, GC, B), BF16,
                              kind="ExternalOutput")
        sall_d = nc.dram_tensor("dbg_sall", (128, NMT * NV2), F32,
                                kind="ExternalOutput")
        lg_d = nc.dram_tensor("dbg_lg", (128, VS), F16, kind="ExternalOutput")
        nlse_d = nc.dram_tensor("dbg_nlse", (128, 1), F32,
                                kind="ExternalOutput")

    with tile.TileContext(nc) as tc:
        with tc.tile_pool(name="sb", bufs=1) as sb, \
             tc.tile_pool(name="ps", bufs=1, space="PSUM") as ps, \
             tc.tile_pool(name="dram", bufs=1, space="DRAM") as dp:

            # ---------------- phase 0 helpers -------------------------------
            xg_tiles = {}

            def emit_prep_gather(c8):
                tlo = 8 * c8
                nst = min(8, S - tlo)
                nrows = B * nst
                xg = sb.tile([128, 8, GC, B], BF16, tag="xg", bufs=2,
                             name=f"xg{c8}")
                xg_tiles[c8] = xg
                embt = sb.tile([128, KE, 256], BF16, tag="embt", bufs=2,
                               name=f"embt{c8}")
                for sub in range(2):
                    lo = tlo * B + sub * 128
                    nr = min(128, nrows - sub * 128)
                    if nr <= 0:
                        continue
                    idx_t = sb.tile([128, 1], I32, tag="idx", bufs=4,
                                    name=f"idx{c8}_{sub}")
                    nc.scalar.dma_start(idx_t[:nr], trg_d[lo:lo + nr, :])
                    rows = sb.tile([128, E], BF16, tag="embr", bufs=4,
                                   name=f"embr{c8}_{sub}")
                    nc.gpsimd.indirect_dma_start(
                        out=rows[:nr], out_offset=None, in_=tbl_d[:],
                        in_offset=bass.IndirectOffsetOnAxis(ap=idx_t[:nr, :1], axis=0))
                    for kb in range(KE):
                        nc.scalar.dma_start_transpose(
                            embt[:, kb, sub * 128:sub * 128 + nr],
                            rows[:nr, kb * 128:(kb + 1) * 128])
                return embt

            def emit_prep_xg(c8, embt, gps):
                # gps: list of even gate-chunk starts; processes pairs
                tlo = 8 * c8
                nst = min(8, S - tlo)
                nrows = B * nst
                xg = xg_tiles[c8]
                for gp in gps:
                    pxg = ps.tile([128, 2, 256], F32, tag="ps_xg", bufs=1,
                                  name=f"pxg{c8}_{gp}")
                    for gi in range(2):
                        gc = gp + gi
                        for kt in range(KE):
                            nc.tensor.matmul(
                                pxg[:, gi, :nrows],
                                lhsT=wih_sb[:, kt, gc * 128:(gc + 1) * 128],
                                rhs=embt[:, kt, :nrows],
                                start=(kt == 0), stop=(kt == KE - 1))
                    src = pxg[:, :, :nrows].rearrange(
                        "p g (t b) -> p t g b", b=B)
                    if bx_nonzero:
                        nc.vector.tensor_tensor(
                            out=xg[:, :nst, gp:gp + 2, :], in0=src,
                            in1=bx_sb[:, None, gp:gp + 2, None].to_broadcast(
                                [128, nst, 2, B]),
                            op=OP.add)
                    elif (gp // 2) % 2 == 0:
                        nc.vector.tensor_copy(xg[:, :nst, gp:gp + 2, :], src)
                    else:
                        nc.scalar.activation(xg[:, :nst, gp:gp + 2, :], src,
                                             AF.Identity)

            # ------- startup: the xg(0) chain first (it's the longest pole),
            # persistent loads next (they overlap the gather/transpose chain).
            embt0 = emit_prep_gather(0)
            h0_sb = sb.tile([128, KH, B], BF16)
            nc.scalar.dma_start(h0_sb[:], h0_d[:])
            whh_sb = sb.tile([128, KH, G], BF16)
            nc.scalar.dma_start(whh_sb[:], whh_d[:])
            wih_sb = sb.tile([128, KE, G], BF16)
            nc.scalar.dma_start(wih_sb[:], wih_d[:])
            bx_sb = sb.tile([128, GC], BF16)
            nc.scalar.dma_start(bx_sb[:], bx_d[:])
            if bhh_n_nonzero:
                bhn_sb = sb.tile([128, KH], BF16)
                nc.scalar.dma_start(bhn_sb[:], bhn_d[:])

            HT = sb.tile([128, KH, NROW], BF16)      # h_{t+1} states, (t, b) cols
            ebias = sb.tile([128, 1], F32)
            nc.gpsimd.memset(ebias[:], EXP_BIAS)
            S_all = sb.tile([128, NMT * NV2], F32)   # exp partial sums
            nc.gpsimd.memset(S_all[:], 0.0)
            ident = sb.tile([128, 128], BF16)
            make_identity(nc, ident[:])

            emit_prep_xg(0, embt0, range(0, GC, 2))
            if debug:
                nc.sync.dma_start(xg_d[:], xg_tiles[0][:])

            # W_proj shard: large, first needed by phase 2 -> load after the
            # startup-critical tensors so it doesn't congest the DMA queues.
            wpr_sb = sb.tile([128, KH, VS], F8)
            nc.sync.dma_start(wpr_sb[:], wpr_d[:])
            if bproj_nonzero:
                bpr_sb = sb.tile([128, VS], F32)
                nc.gpsimd.dma_start(bpr_sb[:], bpr_d[:1, :].to_broadcast([128, VS]))

            # ---------------- phase 1 step ----------------------------------
            def emit_step(t):
                h_prev = h0_sb[:, :, :] if t == 0 else HT[:, :, (t - 1) * B:t * B]
                xg = xg_tiles[t // 8][:, t % 8, :, :]
                # gate chunk layout in one PSUM bank: r 0-3, z 4-7, n 8-11
                pa = ps.tile([128, 12, B], F32, tag="ps_a", bufs=2, name=f"pa{t}")
                # z first (its gps chain z->p is the longest branch),
                # then r (feeds the n chain), then n
                for gc in range(4, 8):
                    for kt in range(KH):
                        nc.tensor.matmul(
                            pa[:, gc, :],
                            lhsT=whh_sb[:, kt, gc * 128:(gc + 1) * 128],
                            rhs=h_prev[:, kt, :],
                            start=(kt == 0), stop=False)
                for gc in range(4, 8):
                    nc.tensor.matmul(
                        pa[:, gc, :], lhsT=ident[:], rhs=xg[:, gc, :],
                        start=False, stop=True)
                for gc in range(4):
                    for kt in range(KH):
                        nc.tensor.matmul(
                            pa[:, gc, :],
                            lhsT=whh_sb[:, kt, gc * 128:(gc + 1) * 128],
                            rhs=h_prev[:, kt, :],
                            start=(kt == 0), stop=False)
                for gc in range(4):
                    nc.tensor.matmul(
                        pa[:, gc, :], lhsT=ident[:], rhs=xg[:, gc, :],
                        start=False, stop=True)
                for gc in range(8, 12):
                    for kt in range(KH):
                        nc.tensor.matmul(
                            pa[:, gc, :],
                            lhsT=whh_sb[:, kt, gc * 128:(gc + 1) * 128],
                            rhs=h_prev[:, kt, :],
                            start=(kt == 0), stop=(kt == KH - 1))
                # z gate first: its sigma feeds the serial gps chain
                zt = sb.tile([128, 4, B], BF16, tag="zt", bufs=2, name=f"zt{t}")
                nc.scalar.activation(zt[:], pa[:, 4:8, :], AF.Tanh, scale=0.5)
                z_s = sb.tile([128, 4, B], BF16, tag="z_s", bufs=2, name=f"zs{t}")
                nc.gpsimd.tensor_scalar(out=z_s[:], in0=zt[:], scalar1=0.5,
                                        scalar2=0.5, op0=OP.mult, op1=OP.add)
                q_s = sb.tile([128, 4, B], BF16, tag="q_s", bufs=2, name=f"qs{t}")
                nc.gpsimd.tensor_scalar(out=q_s[:], in0=zt[:], scalar1=-0.5,
                                        scalar2=0.5, op0=OP.mult, op1=OP.add)
                p_s = sb.tile([128, 4, B], BF16, tag="p_s", bufs=2, name=f"ps{t}")
                nc.gpsimd.tensor_tensor(out=p_s[:], in0=z_s[:], in1=h_prev,
                                        op=OP.mult)
                # r gate: sigma(x) = 0.5*tanh(x/2) + 0.5
                rt = sb.tile([128, 4, B], BF16, tag="rt", bufs=2, name=f"rt{t}")
                nc.scalar.activation(rt[:], pa[:, 0:4, :], AF.Tanh, scale=0.5)
                r_s = sb.tile([128, 4, B], BF16, tag="r_s", bufs=2, name=f"rs{t}")
                nc.vector.tensor_scalar(out=r_s[:], in0=rt[:], scalar1=0.5,
                                        scalar2=0.5, op0=OP.mult, op1=OP.add)
                # n gate
                if bhh_n_nonzero:
                    nc.vector.tensor_tensor(
                        out=pa[:, 8:12, :], in0=pa[:, 8:12, :],
                        in1=bhn_sb[:, :, None].to_broadcast([128, 4, B]), op=OP.add)
                nc.vector.tensor_tensor(out=pa[:, 8:12, :], in0=pa[:, 8:12, :],
                                        in1=r_s[:], op=OP.mult)
                nc.vector.tensor_tensor(out=pa[:, 8:12, :], in0=pa[:, 8:12, :],
                                        in1=xg[:, 8:12, :], op=OP.add)
                n_s = sb.tile([128, 4, B], BF16, tag="n_s", bufs=2, name=f"ns{t}")
                nc.scalar.activation(n_s[:], pa[:, 8:12, :], AF.Tanh)
                # h' = n*(1-z) + z*h
                w_s = sb.tile([128, 4, B], BF16, tag="w_s", bufs=2, name=f"ws{t}")
                nc.vector.tensor_tensor(out=w_s[:], in0=n_s[:], in1=q_s[:],
                                        op=OP.mult)
                nc.vector.tensor_tensor(out=HT[:, :, t * B:(t + 1) * B],
                                        in0=w_s[:], in1=p_s[:], op=OP.add)

            # ---------------- phase 2 emission helpers ----------------------
            logit_tiles = {}
            ht8_tiles = {}
            lse_tiles = {}

            def emit_munit(m, u2):
                # one 1000-vocab double-unit of row-tile m's logits + stats
                mp = min(128, NROW - m * 128)
                if u2 == 0:
                    logit_tiles[m] = sb.tile([128, VS], F16, tag="logit",
                                             bufs=9, name=f"lg{m}")
                    ht8 = sb.tile([128, KH, 128], F8, tag="ht8", bufs=2,
                                  name=f"ht8_{m}")
                    ht8_tiles[m] = ht8
                    nc.vector.tensor_scalar(
                        out=ht8[:, :, :mp], in0=HT[:, :, m * 128:m * 128 + mp],
                        scalar1=HSCL, scalar2=None, op0=OP.mult)
                lg = logit_tiles[m]
                ht8 = ht8_tiles[m]
                pl = ps.tile([128, 2, 512], F32, tag="ps_l", bufs=2,
                             name=f"pl{m}_{u2}")
                for half in range(2):
                    for kp in range(KH // 2):
                        nc.tensor.matmul(
                            pl[:mp, half, :500],
                            lhsT=ht8[:, 2 * kp:2 * kp + 2, :mp],
                            rhs=wpr_sb[:, 2 * kp:2 * kp + 2,
                                       u2 * 1000 + half * 500:
                                       u2 * 1000 + half * 500 + 500],
                            start=(kp == 0), stop=(kp == KH // 2 - 1),
                            perf_mode=DRow)
                src = pl[:mp, :, :500]
                if bproj_nonzero:
                    # bproj_s is pre-scaled by PSCL on the host
                    nc.vector.tensor_tensor(
                        out=src, in0=src,
                        in1=bpr_sb[:mp, u2 * 1000:(u2 + 1) * 1000].rearrange(
                            "p (a b) -> p a b", a=2), op=OP.add)
                # true logits (fp16) parked until the group's lse is known
                nc.vector.tensor_scalar(
                    out=lg[:mp, u2 * 1000:(u2 + 1) * 1000].rearrange(
                        "p (a b) -> p a b", a=2),
                    in0=src, scalar1=1.0 / PSCL, scalar2=None, op0=OP.mult)
                esc = sb.tile([128, 2, 500], F16, tag="exps", bufs=2,
                              name=f"esc{m}_{u2}")
                nc.scalar.activation(
                    esc[:mp], src, AF.Exp, bias=ebias[:mp, :1],
                    scale=1.0 / PSCL,
                    accum_out=S_all[:mp, m * NV2 + u2:m * NV2 + u2 + 1])

            def emit_group_trigger(g):
                # local row-sums + AllGather trigger; readers emitted later
                mlo, cnt = GROUPS[g]
                sg = sb.tile([128, 4], F32, tag="sg", bufs=2, name=f"sg{g}")
                for j in range(cnt):
                    m = mlo + j
                    nc.vector.reduce_sum(
                        out=sg[:, j:j + 1],
                        in_=S_all[:, m * NV2:(m + 1) * NV2],
                        axis=mybir.AxisListType.X)
                cin = dp.tile([128, cnt], F32, tag=f"cin{g}", name=f"cin{g}")
                nc.gpsimd.dma_start(cin[:], sg[:, :cnt])
                cout = dp.tile([NCORES, 128, cnt], F32, tag=f"cout{g}",
                               addr_space="Shared", name=f"cout{g}")
                nc.gpsimd.collective_compute(
                    "AllGather", OP.bypass,
                    replica_groups=[list(range(NCORES))],
                    ins=[cin.opt()], outs=[cout.opt()])
                return cout

            def emit_group_stats(g, cout):
                mlo, cnt = GROUPS[g]
                s8 = sb.tile([128, cnt, NCORES], F32, tag="s8", bufs=2,
                             name=f"s8{g}")
                nc.sync.dma_start(s8[:], cout[:].rearrange("c p m -> p m c"))
                st = sb.tile([128, cnt], F32, tag="st", bufs=2, name=f"st{g}")
                nc.vector.reduce_sum(out=st[:], in_=s8[:],
                                     axis=mybir.AxisListType.X)
                # neg_lse = -(e - 127 + 4) * ln2 - ln(m),  St = m * 2^(e-127)
                iu = st[:].bitcast(U32)
                eu = sb.tile([128, cnt], U32, tag="eu", bufs=2, name=f"eu{g}")
                nc.vector.tensor_scalar(out=eu[:], in0=iu, scalar1=23,
                                        scalar2=None, op0=OP.logical_shift_right)
                ef = sb.tile([128, cnt], F32, tag="ef", bufs=2, name=f"ef{g}")
                nc.vector.tensor_copy(ef[:], eu[:])
                mu = sb.tile([128, cnt], U32, tag="mu", bufs=2, name=f"mu{g}")
                nc.vector.tensor_scalar(out=mu[:], in0=iu, scalar1=0x007FFFFF,
                                        scalar2=0x3F800000, op0=OP.bitwise_and,
                                        op1=OP.bitwise_or)
                mf = mu[:].bitcast(F32)
                acc = sb.tile([128, cnt], F32, tag="acc", bufs=2, name=f"acc{g}")
                c = _NEGLN_COEF
                nc.vector.tensor_scalar(out=acc[:], in0=mf, scalar1=c[0],
                                        scalar2=c[1], op0=OP.mult, op1=OP.add)
                for k in range(2, 6):
                    nc.vector.tensor_tensor(out=acc[:], in0=acc[:], in1=mf,
                                            op=OP.mult)
                    nc.vector.tensor_scalar(out=acc[:], in0=acc[:], scalar1=c[k],
                                            scalar2=None, op0=OP.add)
                # + (127 - 4 - e) * ln2   (the -4 re-adds the exp bias so
                #   lse refers to unshifted logits)
                e2 = sb.tile([128, cnt], F32, tag="e2", bufs=2, name=f"e2{g}")
                nc.vector.tensor_scalar(out=e2[:], in0=ef[:], scalar1=-LN2,
                                        scalar2=(127.0 - 4.0) * LN2,
                                        op0=OP.mult, op1=OP.add)
                nlse = sb.tile([128, cnt], F32, tag="nlse", bufs=2,
                               name=f"nlse{g}")
                nc.vector.tensor_tensor(out=nlse[:], in0=acc[:], in1=e2[:],
                                        op=OP.add)
                lse_tiles[g] = nlse
                if debug and g == 0:
                    nc.sync.dma_start(nlse_d[:], nlse[:])

            def emit_output(m):
                g = GRP_OF_M[m]
                j = m - GROUPS[g][0]
                mp = min(128, NROW - m * 128)
                nlse = lse_tiles[g]
                lg = logit_tiles.pop(m)
                ht8_tiles.pop(m, None)
                if debug and m == 0:
                    nc.sync.dma_start(lg_d[:], lg[:])
                ot = sb.tile([128, VS], F16, tag="ot", bufs=2, name=f"ot{m}")
                nc.vector.tensor_scalar(out=ot[:mp], in0=lg[:mp],
                                        scalar1=nlse[:mp, j:j + 1],
                                        scalar2=None, op0=OP.add)
                nc.sync.dma_start(out_d[m * 128:m * 128 + mp, :], ot[:mp])

            # ---------------- main emission loop ----------------------------
            # Interleave prep / phase-2 work between steps in small pieces so
            # the scheduler can't starve the serial recurrence on PE.  Items
            # that read an AllGather result are released several drain slots
            # after the trigger so no engine queue blocks on AG latency; the
            # per-group outputs release one per slot to avoid bursts.
            from collections import deque
            work_q = deque()
            delayed = []          # (release_tick, fn), kept sorted
            tick = 0

            def fin_ready(g):
                def trig(g=g):
                    cout = emit_group_trigger(g)
                    rel = tick + GDELAY[g]
                    delayed.append((rel, lambda g=g, cout=cout:
                                    emit_group_stats(g, cout)))
                    mlo, cnt = GROUPS[g]
                    for k in range(cnt):
                        delayed.append((rel + 1 + k,
                                        lambda m=mlo + k: emit_output(m)))
                    delayed.sort(key=lambda x: x[0])
                return trig

            def enqueue_mtile(m):
                for u2 in range(NV2):
                    work_q.append(lambda m=m, u2=u2: emit_munit(m, u2))
                for g, (mlo, cnt) in enumerate(GROUPS):
                    if m == mlo + cnt - 1:
                        work_q.append(fin_ready(g))

            def drain(n):
                nonlocal tick
                for _ in range(n):
                    tick += 1
                    if delayed and delayed[0][0] <= tick:
                        delayed.pop(0)[1]()
                    elif work_q:
                        work_q.popleft()()
                    else:
                        break

            for t in range(S):
                emit_step(t)
                if t % 8 == 1 and t // 8 + 1 <= (S - 1) // 8:
                    c8 = t // 8 + 1
                    embt = emit_prep_gather(c8)
                    for lo in range(0, GC, 4):
                        work_q.append(lambda c8=c8, embt=embt, lo=lo:
                                      emit_prep_xg(c8, embt, range(lo, lo + 4, 2)))
                if t >= 3 and (t - 3) % 4 == 0:
                    enqueue_mtile((t - 3) // 4)
                drain(3)
            for m in range(((S - 1 - 3) // 4) + 1, NMT):
                enqueue_mtile(m)
            while work_q or delayed:
                if not work_q and delayed:
                    tick = max(tick, delayed[0][0] - 1)
                drain(1)
            if debug:
                nc.sync.dma_start(ht_d[:], HT[:])
                nc.sync.dma_start(sall_d[:], S_all[:])

    nc.finalize()
    _BUILD_CACHE[key] = nc
    return nc


def _pack_T(w, ktiles, dtype=ml_dtypes.bfloat16, scale=1.0):
    """[out_dim, in_dim] f32 -> [128, ktiles, out_dim] (w.T, k-major slabs)."""
    wT = np.ascontiguousarray(w.T * scale).astype(dtype)
    return np.ascontiguousarray(
        wT.reshape(ktiles, 128, w.shape[0]).transpose(1, 0, 2))


LAST_PROFILE = None


def kernel(trg, h0, embed_table, W_ih, W_hh, b_ih, b_hh, W_proj, b_proj):
    global LAST_PROFILE
    trg = np.asarray(trg)
    h0 = np.asarray(h0, dtype=np.float32)
    embed_table = np.asarray(embed_table, dtype=np.float32)
    W_ih = np.asarray(W_ih, dtype=np.float32)
    W_hh = np.asarray(W_hh, dtype=np.float32)
    b_ih = np.asarray(b_ih, dtype=np.float32)
    b_hh = np.asarray(b_hh, dtype=np.float32)
    W_proj = np.asarray(W_proj, dtype=np.float32)
    b_proj = np.asarray(b_proj, dtype=np.float32)

    # bx = b_ih + [b_hh for r,z chunks; 0 for n chunks], packed [128, GC]
    bx = b_ih.copy()
    bx[:2 * H] += b_hh[:2 * H]
    bhh_n_nonzero = bool(np.any(b_hh[2 * H:]))
    bproj_nonzero = bool(np.any(b_proj))
    bx_nonzero = bool(np.any(bx))
    nc = _build(bhh_n_nonzero, bproj_nonzero, bx_nonzero)

    # host-side layout prep (sharding/packing only)
    trg_flat = np.ascontiguousarray(
        trg[:, :S].T.reshape(NROW, 1)).astype(np.int32)
    tbl_bf = embed_table.astype(ml_dtypes.bfloat16)
    wih_t = _pack_T(W_ih, KE)
    whh_t = _pack_T(W_hh, KH)
    h0_t = np.ascontiguousarray(
        h0[0].T.reshape(KH, 128, B).transpose(1, 0, 2)).astype(ml_dtypes.bfloat16)
    bx_t = np.ascontiguousarray(bx.reshape(GC, 128).T).astype(ml_dtypes.bfloat16)

    base = {
        "trg_flat": trg_flat,
        "emb_tbl": tbl_bf,
        "wih_t": wih_t,
        "whh_t": whh_t,
        "h0_t": h0_t,
        "bx_t": bx_t,
    }
    if bhh_n_nonzero:
        base["bhn_t"] = np.ascontiguousarray(
            b_hh[2 * H:].reshape(KH, 128).T).astype(ml_dtypes.bfloat16)

    in_maps = []
    for c in range(NCORES):
        m = dict(base)
        m["wproj_t"] = _pack_T(W_proj[c * VS:(c + 1) * VS], KH,
                               dtype=ml_dtypes.float8_e4m3, scale=WSCL)
        if bproj_nonzero:
            m["bproj_s"] = np.ascontiguousarray(
                (b_proj[c * VS:(c + 1) * VS] * PSCL).reshape(1, VS))
        in_maps.append(m)

    trace = bool(int(os.environ.get("KERNEL_TRACE", "0")))
    res = run_bass_kernel_spmd(nc, in_maps, core_ids=list(range(NCORES)),
                               trace=trace)
    LAST_PROFILE = res

    out = np.zeros((B, T, V), dtype=np.float32)
    big = np.stack([res.results[c]["out_lp"].reshape(S, B, VS)
                    for c in range(NCORES)], axis=0)   # [c, t, b, vs]
    out[:, 1:, :] = big.transpose(2, 1, 0, 3).reshape(B, S, V)
    return out


# revision 10
# speedup vs baseline: 1.4198x; 1.2658x over previous
"""GRU decoder (teacher forcing) + log_softmax on 8 Trainium2 NeuronCores.

Strategy (v3):
  - Vocab-shard the projection/log-softmax across the 8 cores (W_proj rows),
    replicate the (tiny, serial) GRU recurrence on every core.
  - Phase 0 (per 8-step chunk): indirect-DMA gather of embedding rows,
    DMA-xbar transpose to k-major, matmul -> x-side gate pre-activations
    XG = emb @ W_ih.T (+ b_ih + b_hh[r,z]) stored time-major in SBUF.
  - Phase 1 (63 sequential steps): hg^T = W_hh^T-slabs.T @ h^T on PE
    (weights stationary, batch on the moving free axis); all 12 gate chunks
    share one PSUM bank; tanh(r) fires as soon as the r chunks are done so
    the n-gate chain starts early; z is finished while n is in flight.
    sigmoid(x) = 0.5*tanh(x/2)+0.5 so only the exp_and_others ACT table is
    ever loaded (tanh+exp+identity live there; no table switches).
  - Phase 2 (16 row-tiles of 128): logits = HT-slabs.T @ W_projT-shard with
    fp8e4 DoubleRow matmuls (W_proj pre-scaled x64 on host, h x8 on device;
    PSUM holds 512*logit in [128,2,512] double-unit tiles).  One exp
    (scale=1/512, bias=-4ln2) per 1000 cols with accum_out row partial
    sums; true logits parked in fp16 SBUF rings via one DVE scale per
    1000 cols.  Small AllGathers exchange partial sums per group of
    row-tiles; group 0 is a single row-tile so the first (slow, path-
    warming) collective fires early with its readers delayed far enough
    to never block an engine queue.  lse via DVE bit-twiddle log; final
    out = logit - lse on DVE (4x fp16 tensor_scalar, per-row AP scalar),
    DMA'd as fp16 and upcast on host.

kernel(**inputs) takes the FULL numpy inputs, does layout prep on host,
runs the SPMD NEFF on cores 0..7 and reassembles the [32, 64, 32000] output.
"""

import os

import numpy as np
import ml_dtypes

import concourse.bass as bass
import concourse.bacc as bacc
import concourse.mybir as mybir
import concourse.tile as tile
from concourse.bass_utils import run_bass_kernel_spmd
from concourse.masks import make_identity

# problem shape (hardcoded per contract)
B, T, V, E, H = 32, 64, 32000, 256, 512
S = T - 1                 # 63 decode steps
NCORES = 8
VS = V // NCORES          # 4000 vocab shard per core
G = 3 * H                 # 1536 gate dims
GC = G // 128             # 12 gate chunks
KH = H // 128             # 4 contraction tiles over H
KE = E // 128             # 2 contraction tiles over E
NROW = S * B              # 2016 output rows, (t, b) order
NMT = (NROW + 127) // 128  # 16 row-tiles (last has 96 rows)
# stat-collective groups (start_mtile, count) and reader release delays
GROUPS = [(0, 1), (1, 3), (4, 4), (8, 4), (12, 3), (15, 1)]
GDELAY = [14, 10, 10, 10, 8, 3]
GRP_OF_M = {}
for _gi, (_s, _c) in enumerate(GROUPS):
    for _m in range(_s, _s + _c):
        GRP_OF_M[_m] = _gi
NV2 = VS // 1000          # 4 double-units (1000 vocab) per row-tile
LN2 = float(np.log(2.0))
EXP_BIAS = -4.0 * LN2     # exp(logit - 4ln2): keeps fp16 exp safely in range
WSCL = 64.0               # host pre-scale of W_proj before fp8 cast
HSCL = 8.0                # device pre-scale of h before fp8 cast
PSCL = WSCL * HSCL        # PSUM = PSCL * logit

F32 = mybir.dt.float32
BF16 = mybir.dt.bfloat16
F16 = mybir.dt.float16
F8 = mybir.dt.float8e4
I32 = mybir.dt.int32
U32 = mybir.dt.uint32
AF = mybir.ActivationFunctionType
OP = mybir.AluOpType
DRow = mybir.MatmulPerfMode.DoubleRow

# -ln(m) Chebyshev-interpolation coefficients on m in [1, 2], highest first.
_nodes = np.cos((2 * np.arange(1, 7) - 1) / (2 * 6.0) * np.pi) * 0.5 + 1.5
_NEGLN_COEF = [float(c) for c in np.polyfit(_nodes, -np.log(_nodes), 5)]

_BUILD_CACHE = {}


def _build(bhh_n_nonzero: bool, bproj_nonzero: bool, bx_nonzero: bool):
    debug = bool(int(os.environ.get("KERNEL_DEBUG", "0")))
    key = (bhh_n_nonzero, bproj_nonzero, bx_nonzero, debug)
    if key in _BUILD_CACHE:
        return _BUILD_CACHE[key]

    nc = bacc.Bacc("TRN2", target_bir_lowering=False, debug=False,
                   enable_asserts=False, num_devices=NCORES)

    trg_d = nc.dram_tensor("trg_flat", (NROW, 1), I32, kind="ExternalInput")
    tbl_d = nc.dram_tensor("emb_tbl", (V, E), BF16, kind="ExternalInput")
    wih_d = nc.dram_tensor("wih_t", (128, KE, G), BF16, kind="ExternalInput")
    whh_d = nc.dram_tensor("whh_t", (128, KH, G), BF16, kind="ExternalInput")
    h0_d = nc.dram_tensor("h0_t", (128, KH, B), BF16, kind="ExternalInput")
    wpr_d = nc.dram_tensor("wproj_t", (128, KH, VS), F8, kind="ExternalInput")
    bx_d = nc.dram_tensor("bx_t", (128, GC), BF16, kind="ExternalInput")
    if bhh_n_nonzero:
        bhn_d = nc.dram_tensor("bhn_t", (128, KH), BF16, kind="ExternalInput")
    if bproj_nonzero:
        bpr_d = nc.dram_tensor("bproj_s", (1, VS), F32, kind="ExternalInput")
    out_d = nc.dram_tensor("out_lp", (NROW, VS), F16, kind="ExternalOutput")
    if debug:
        ht_d = nc.dram_tensor("dbg_ht", (128, KH, NROW), BF16,
                              kind="ExternalOutput")
        xg_d = nc.dram_tensor("dbg_xg", (128, 8, GC, B), BF16,
                              kind="ExternalOutput")
        sall_d = nc.dram_tensor("dbg_sall", (128, NMT * NV2), F32,
                                kind="ExternalOutput")
        lg_d = nc.dram_tensor("dbg_lg", (128, VS), F16, kind="ExternalOutput")
        nlse_d = nc.dram_tensor("dbg_nlse", (128, 1), F32,
                                kind="ExternalOutput")

    with tile.TileContext(nc) as tc:
        with tc.tile_pool(name="sb", bufs=1) as sb, \
             tc.tile_pool(name="ps", bufs=1, space="PSUM") as ps, \
             tc.tile_pool(name="dram", bufs=1, space="DRAM") as dp:

            # ---------------- phase 0 helpers -------------------------------
            xg_tiles = {}
            embt_tiles = {}

            rows_tiles = {}

            def emit_prep_gather(c8):
                tlo = 8 * c8
                nst = min(8, S - tlo)
                nrows = B * nst
                xg = sb.tile([128, 8, GC, B], BF16, tag="xg", bufs=2,
                             name=f"xg{c8}")
                xg_tiles[c8] = xg
                embt = sb.tile([128, KE, 256], BF16, tag="embt", bufs=2,
                               name=f"embt{c8}")
                embt_tiles[c8] = embt
                for sub in range(2):
                    lo = tlo * B + sub * 128
                    nr = min(128, nrows - sub * 128)
                    if nr <= 0:
                        continue
                    idx_t = sb.tile([128, 1], I32, tag="idx", bufs=4,
                                    name=f"idx{c8}_{sub}")
                    nc.sync.dma_start(idx_t[:nr], trg_d[lo:lo + nr, :])
                    rows = sb.tile([128, E], BF16, tag="embr", bufs=4,
                                   name=f"embr{c8}_{sub}")
                    rows_tiles[(c8, sub)] = (rows, nr)
                    nc.gpsimd.indirect_dma_start(
                        out=rows[:nr], out_offset=None, in_=tbl_d[:],
                        in_offset=bass.IndirectOffsetOnAxis(ap=idx_t[:nr, :1], axis=0))
                return embt

            def emit_prep_tr(c8):
                # PE transpose (not DMA transpose: the tile framework
                # serializes DMA transposes with collectives, which would
                # put every AllGather on the xg critical path)
                embt = embt_tiles[c8]
                for sub in range(2):
                    if (c8, sub) not in rows_tiles:
                        continue
                    rows, nr = rows_tiles.pop((c8, sub))
                    tr = ps.tile([128, KE, 128], BF16, tag="ps_tr", bufs=1,
                                 name=f"tr{c8}_{sub}")
                    for kb in range(KE):
                        nc.tensor.matmul(
                            tr[:, kb, :nr], lhsT=rows[:nr, kb * 128:(kb + 1) * 128],
                            rhs=ident[:nr, :nr], is_transpose=True)
                    nc.vector.tensor_copy(
                        embt[:, :, sub * 128:sub * 128 + nr], tr[:, :, :nr])

            def emit_prep_xg(c8, embt, gps):
                # gps: list of even gate-chunk starts; processes pairs
                tlo = 8 * c8
                nst = min(8, S - tlo)
                nrows = B * nst
                xg = xg_tiles[c8]
                for gp in gps:
                    pxg = ps.tile([128, 2, 256], F32, tag="ps_xg", bufs=1,
                                  name=f"pxg{c8}_{gp}")
                    for gi in range(2):
                        gc = gp + gi
                        for kt in range(KE):
                            nc.tensor.matmul(
                                pxg[:, gi, :nrows],
                                lhsT=wih_sb[:, kt, gc * 128:(gc + 1) * 128],
                                rhs=embt[:, kt, :nrows],
                                start=(kt == 0), stop=(kt == KE - 1))
                    src = pxg[:, :, :nrows].rearrange(
                        "p g (t b) -> p t g b", b=B)
                    if bx_nonzero:
                        nc.vector.tensor_tensor(
                            out=xg[:, :nst, gp:gp + 2, :], in0=src,
                            in1=bx_sb[:, None, gp:gp + 2, None].to_broadcast(
                                [128, nst, 2, B]),
                            op=OP.add)
                    elif (gp // 2) % 2 == 0:
                        nc.vector.tensor_copy(xg[:, :nst, gp:gp + 2, :], src)
                    else:
                        nc.scalar.activation(xg[:, :nst, gp:gp + 2, :], src,
                                             AF.Identity)

            # ------- startup ------------------------------------------------
            HT = sb.tile([128, KH, NROW], BF16)      # h_{t+1} states, (t, b) cols
            ebias = sb.tile([128, 1], F32)
            nc.gpsimd.memset(ebias[:], EXP_BIAS)
            S_all = sb.tile([128, NMT * NV2], F32)   # exp partial sums
            nc.gpsimd.memset(S_all[:], 0.0)
            # warm up the collective path on the Scalar queue: nothing else
            # DMA-shaped lives there, so the slow first collective can't
            # fence any compute-critical transfer behind it.
            warm_in = dp.tile([128, 1], F32, tag="warm_in")
            warm_out = dp.tile([NCORES, 128, 1], F32, tag="warm_out",
                               addr_space="Shared")
            nc.scalar.dma_start(warm_in[:], ebias[:])
            nc.gpsimd.collective_compute(
                "AllGather", OP.bypass, replica_groups=[list(range(NCORES))],
                ins=[warm_in.opt()], outs=[warm_out.opt()])
            # recurrence-critical loads, then the xg(0) gather chain
            h0_sb = sb.tile([128, KH, B], BF16)
            nc.sync.dma_start(h0_sb[:], h0_d[:])
            whh_sb = sb.tile([128, KH, G], BF16)
            nc.sync.dma_start(whh_sb[:], whh_d[:])
            wih_sb = sb.tile([128, KE, G], BF16)
            nc.sync.dma_start(wih_sb[:], wih_d[:])
            bx_sb = sb.tile([128, GC], BF16)
            nc.sync.dma_start(bx_sb[:], bx_d[:])
            if bhh_n_nonzero:
                bhn_sb = sb.tile([128, KH], BF16)
                nc.sync.dma_start(bhn_sb[:], bhn_d[:])
            ident = sb.tile([128, 128], BF16)
            make_identity(nc, ident[:])
            embt0 = emit_prep_gather(0)
            emit_prep_tr(0)
            emit_prep_xg(0, embt0, range(0, GC, 2))
            if debug:
                nc.sync.dma_start(xg_d[:], xg_tiles[0][:])

            # W_proj shard: large, first needed by phase 2 -> load after the
            # startup-critical tensors so it doesn't congest the DMA queues.
            wpr_sb = sb.tile([128, KH, VS], F8)
            nc.sync.dma_start(wpr_sb[:], wpr_d[:])
            if bproj_nonzero:
                bpr_sb = sb.tile([128, VS], F32)
                nc.gpsimd.dma_start(bpr_sb[:], bpr_d[:1, :].to_broadcast([128, VS]))

            # ---------------- phase 1 step ----------------------------------
            def emit_step_mm(t):
                h_prev = h0_sb[:, :, :] if t == 0 else HT[:, :, (t - 1) * B:t * B]
                xg = xg_tiles[t // 8][:, t % 8, :, :]
                # r,z gate chunks share one PSUM tile; n separate so its
                # late matmuls don't gate the rz tanh (tile-granular deps).
                rz = ps.tile([128, 8, B], F32, tag="ps_rz", bufs=1,
                             name=f"rz{t}")
                pn = ps.tile([128, 4, B], F32, tag="ps_n", bufs=1,
                             name=f"pn{t}")
                for gc in range(8):
                    for kt in range(KH):
                        nc.tensor.matmul(
                            rz[:, gc, :],
                            lhsT=whh_sb[:, kt, gc * 128:(gc + 1) * 128],
                            rhs=h_prev[:, kt, :],
                            start=(kt == 0), stop=False)
                for gc in range(8):
                    nc.tensor.matmul(
                        rz[:, gc, :], lhsT=ident[:], rhs=xg[:, gc, :],
                        start=False, stop=True)
                for gc in range(4):
                    for kt in range(KH):
                        nc.tensor.matmul(
                            pn[:, gc, :],
                            lhsT=whh_sb[:, kt, (8 + gc) * 128:(9 + gc) * 128],
                            rhs=h_prev[:, kt, :],
                            start=(kt == 0), stop=(kt == KH - 1))
                return rz, pn

            def emit_step_gates(t, rz, pn):
                h_prev = h0_sb[:, :, :] if t == 0 else HT[:, :, (t - 1) * B:t * B]
                xg = xg_tiles[t // 8][:, t % 8, :, :]
                # r,z gates: sigma(x) = 0.5*tanh(x/2) + 0.5 in two fused ops
                rzt = sb.tile([128, 8, B], BF16, tag="rzt", bufs=2,
                              name=f"rzt{t}")
                nc.scalar.activation(rzt[:], rz[:], AF.Tanh, scale=0.5)
                rz_s = sb.tile([128, 8, B], BF16, tag="rz_s", bufs=2,
                               name=f"rzs{t}")
                nc.vector.tensor_scalar(out=rz_s[:], in0=rzt[:], scalar1=0.5,
                                        scalar2=0.5, op0=OP.mult, op1=OP.add)
                q_s = sb.tile([128, 4, B], BF16, tag="q_s", bufs=2, name=f"qs{t}")
                nc.gpsimd.tensor_scalar(out=q_s[:], in0=rzt[:, 4:8, :],
                                        scalar1=-0.5, scalar2=0.5,
                                        op0=OP.mult, op1=OP.add)
                p_s = sb.tile([128, 4, B], BF16, tag="p_s", bufs=2, name=f"ps{t}")
                nc.gpsimd.tensor_tensor(out=p_s[:], in0=rz_s[:, 4:8, :],
                                        in1=h_prev, op=OP.mult)
                # n gate
                if bhh_n_nonzero:
                    nc.vector.tensor_tensor(
                        out=pn[:], in0=pn[:],
                        in1=bhn_sb[:, :, None].to_broadcast([128, 4, B]), op=OP.add)
                nc.vector.tensor_tensor(out=pn[:], in0=pn[:],
                                        in1=rz_s[:, 0:4, :], op=OP.mult)
                nc.vector.tensor_tensor(out=pn[:], in0=pn[:],
                                        in1=xg[:, 8:12, :], op=OP.add)
                n_s = sb.tile([128, 4, B], BF16, tag="n_s", bufs=2, name=f"ns{t}")
                nc.scalar.activation(n_s[:], pn[:], AF.Tanh)
                # h' = n*(1-z) + z*h
                w_s = sb.tile([128, 4, B], BF16, tag="w_s", bufs=2, name=f"ws{t}")
                nc.vector.tensor_tensor(out=w_s[:], in0=n_s[:], in1=q_s[:],
                                        op=OP.mult)
                nc.vector.tensor_tensor(out=HT[:, :, t * B:(t + 1) * B],
                                        in0=w_s[:], in1=p_s[:], op=OP.add)

            # ---------------- phase 2 emission helpers ----------------------
            logit_tiles = {}
            ht8_tiles = {}
            lse_tiles = {}

            def emit_munit(m, u2):
                # one 1000-vocab double-unit of row-tile m's logits + stats
                mp = min(128, NROW - m * 128)
                if u2 == 0:
                    logit_tiles[m] = sb.tile([128, VS], F16, tag="logit",
                                             bufs=9, name=f"lg{m}")
                    ht8 = sb.tile([128, KH, 128], F8, tag="ht8", bufs=2,
                                  name=f"ht8_{m}")
                    ht8_tiles[m] = ht8
                    nc.vector.tensor_scalar(
                        out=ht8[:, :, :mp], in0=HT[:, :, m * 128:m * 128 + mp],
                        scalar1=HSCL, scalar2=None, op0=OP.mult)
                lg = logit_tiles[m]
                ht8 = ht8_tiles[m]
                pl = ps.tile([128, 2, 512], F32, tag="ps_l", bufs=2,
                             name=f"pl{m}_{u2}")
                for half in range(2):
                    for kp in range(KH // 2):
                        nc.tensor.matmul(
                            pl[:mp, half, :500],
                            lhsT=ht8[:, 2 * kp:2 * kp + 2, :mp],
                            rhs=wpr_sb[:, 2 * kp:2 * kp + 2,
                                       u2 * 1000 + half * 500:
                                       u2 * 1000 + half * 500 + 500],
                            start=(kp == 0), stop=(kp == KH // 2 - 1),
                            perf_mode=DRow)
                src = pl[:mp, :, :500]
                if bproj_nonzero:
                    # bproj_s is pre-scaled by PSCL on the host
                    nc.vector.tensor_tensor(
                        out=src, in0=src,
                        in1=bpr_sb[:mp, u2 * 1000:(u2 + 1) * 1000].rearrange(
                            "p (a b) -> p a b", a=2), op=OP.add)
                # true logits (fp16) parked until the group's lse is known
                nc.vector.tensor_scalar(
                    out=lg[:mp, u2 * 1000:(u2 + 1) * 1000].rearrange(
                        "p (a b) -> p a b", a=2),
                    in0=src, scalar1=1.0 / PSCL, scalar2=None, op0=OP.mult)
                esc = sb.tile([128, 2, 500], F16, tag="exps", bufs=2,
                              name=f"esc{m}_{u2}")
                nc.scalar.activation(
                    esc[:mp], src, AF.Exp, bias=ebias[:mp, :1],
                    scale=1.0 / PSCL,
                    accum_out=S_all[:mp, m * NV2 + u2:m * NV2 + u2 + 1])

            def emit_group_trigger(g):
                # local row-sums + AllGather trigger; readers emitted later
                mlo, cnt = GROUPS[g]
                sg = sb.tile([128, 4], F32, tag="sg", bufs=2, name=f"sg{g}")
                for j in range(cnt):
                    m = mlo + j
                    nc.vector.reduce_sum(
                        out=sg[:, j:j + 1],
                        in_=S_all[:, m * NV2:(m + 1) * NV2],
                        axis=mybir.AxisListType.X)
                cin = dp.tile([128, cnt], F32, tag=f"cin{g}", name=f"cin{g}")
                nc.gpsimd.dma_start(cin[:], sg[:, :cnt])
                cout = dp.tile([NCORES, 128, cnt], F32, tag=f"cout{g}",
                               addr_space="Shared", name=f"cout{g}")
                nc.gpsimd.collective_compute(
                    "AllGather", OP.bypass,
                    replica_groups=[list(range(NCORES))],
                    ins=[cin.opt()], outs=[cout.opt()])
                return cout

            def emit_group_stats(g, cout):
                mlo, cnt = GROUPS[g]
                s8 = sb.tile([128, cnt, NCORES], F32, tag="s8", bufs=2,
                             name=f"s8{g}")
                nc.sync.dma_start(s8[:], cout[:].rearrange("c p m -> p m c"))
                st = sb.tile([128, cnt], F32, tag="st", bufs=2, name=f"st{g}")
                nc.vector.reduce_sum(out=st[:], in_=s8[:],
                                     axis=mybir.AxisListType.X)
                # neg_lse = -(e - 127 + 4) * ln2 - ln(m),  St = m * 2^(e-127)
                iu = st[:].bitcast(U32)
                eu = sb.tile([128, cnt], U32, tag="eu", bufs=2, name=f"eu{g}")
                nc.vector.tensor_scalar(out=eu[:], in0=iu, scalar1=23,
                                        scalar2=None, op0=OP.logical_shift_right)
                ef = sb.tile([128, cnt], F32, tag="ef", bufs=2, name=f"ef{g}")
                nc.vector.tensor_copy(ef[:], eu[:])
                mu = sb.tile([128, cnt], U32, tag="mu", bufs=2, name=f"mu{g}")
                nc.vector.tensor_scalar(out=mu[:], in0=iu, scalar1=0x007FFFFF,
                                        scalar2=0x3F800000, op0=OP.bitwise_and,
                                        op1=OP.bitwise_or)
                mf = mu[:].bitcast(F32)
                acc = sb.tile([128, cnt], F32, tag="acc", bufs=2, name=f"acc{g}")
                c = _NEGLN_COEF
                nc.vector.tensor_scalar(out=acc[:], in0=mf, scalar1=c[0],
                                        scalar2=c[1], op0=OP.mult, op1=OP.add)
                for k in range(2, 6):
                    nc.vector.tensor_tensor(out=acc[:], in0=acc[:], in1=mf,
                                            op=OP.mult)
                    nc.vector.tensor_scalar(out=acc[:], in0=acc[:], scalar1=c[k],
                                            scalar2=None, op0=OP.add)
                # + (127 - 4 - e) * ln2   (the -4 re-adds the exp bias so
                #   lse refers to unshifted logits)
                e2 = sb.tile([128, cnt], F32, tag="e2", bufs=2, name=f"e2{g}")
                nc.vector.tensor_scalar(out=e2[:], in0=ef[:], scalar1=-LN2,
                                        scalar2=(127.0 - 4.0) * LN2,
                                        op0=OP.mult, op1=OP.add)
                nlse = sb.tile([128, cnt], F32, tag="nlse", bufs=2,
                               name=f"nlse{g}")
                nc.vector.tensor_tensor(out=nlse[:], in0=acc[:], in1=e2[:],
                                        op=OP.add)
                lse_tiles[g] = nlse
                if debug and g == 0:
                    nc.sync.dma_start(nlse_d[:], nlse[:])

            def emit_output(m):
                g = GRP_OF_M[m]
                j = m - GROUPS[g][0]
                mp = min(128, NROW - m * 128)
                nlse = lse_tiles[g]
                lg = logit_tiles.pop(m)
                ht8_tiles.pop(m, None)
                if debug and m == 0:
                    nc.sync.dma_start(lg_d[:], lg[:])
                ot = sb.tile([128, VS], F16, tag="ot", bufs=2, name=f"ot{m}")
                nc.vector.tensor_scalar(out=ot[:mp], in0=lg[:mp],
                                        scalar1=nlse[:mp, j:j + 1],
                                        scalar2=None, op0=OP.add)
                nc.sync.dma_start(out_d[m * 128:m * 128 + mp, :], ot[:mp])

            # ---------------- main emission loop ----------------------------
            # Interleave prep / phase-2 work between steps in small pieces so
            # the scheduler can't starve the serial recurrence on PE.  Items
            # that read an AllGather result are released several drain slots
            # after the trigger so no engine queue blocks on AG latency; the
            # per-group outputs release one per slot to avoid bursts.
            from collections import deque
            work_q = deque()
            delayed = []          # (release_tick, fn), kept sorted
            tick = 0

            def fin_ready(g):
                def trig(g=g):
                    cout = emit_group_trigger(g)
                    rel = tick + GDELAY[g]
                    delayed.append((rel, lambda g=g, cout=cout:
                                    emit_group_stats(g, cout)))
                    mlo, cnt = GROUPS[g]
                    for k in range(cnt):
                        delayed.append((rel + 2 + 2 * k,
                                        lambda m=mlo + k: emit_output(m)))
                    delayed.sort(key=lambda x: x[0])
                return trig

            def enqueue_mtile(m):
                for u2 in range(NV2):
                    work_q.append(lambda m=m, u2=u2: emit_munit(m, u2))
                for g, (mlo, cnt) in enumerate(GROUPS):
                    if m == mlo + cnt - 1:
                        work_q.append(fin_ready(g))

            def drain(n):
                nonlocal tick
                for _ in range(n):
                    tick += 1
                    if delayed and delayed[0][0] <= tick:
                        delayed.pop(0)[1]()
                    elif work_q:
                        work_q.popleft()()
                    else:
                        break

            rzpn = emit_step_mm(0)
            for t in range(S):
                emit_step_gates(t, *rzpn)
                if t + 1 < S:
                    rzpn = emit_step_mm(t + 1)
                if t % 8 == 1 and t // 8 + 1 <= (S - 1) // 8:
                    c8 = t // 8 + 1
                    embt = emit_prep_gather(c8)
                    work_q.append(lambda c8=c8: emit_prep_tr(c8))
                    for lo in range(0, GC, 4):
                        work_q.append(lambda c8=c8, embt=embt, lo=lo:
                                      emit_prep_xg(c8, embt, range(lo, lo + 4, 2)))
                if t >= 3 and (t - 3) % 4 == 0:
                    enqueue_mtile((t - 3) // 4)
                drain(3)
            for m in range(((S - 1 - 3) // 4) + 1, NMT):
                enqueue_mtile(m)
            while work_q or delayed:
                if not work_q and delayed:
                    tick = max(tick, delayed[0][0] - 1)
                drain(1)
            if debug:
                nc.sync.dma_start(ht_d[:], HT[:])
                nc.sync.dma_start(sall_d[:], S_all[:])

    nc.finalize()
    _BUILD_CACHE[key] = nc
    return nc


def _pack_T(w, ktiles, dtype=ml_dtypes.bfloat16, scale=1.0):
    """[out_dim, in_dim] f32 -> [128, ktiles, out_dim] (w.T, k-major slabs)."""
    wT = np.ascontiguousarray(w.T * scale).astype(dtype)
    return np.ascontiguousarray(
        wT.reshape(ktiles, 128, w.shape[0]).transpose(1, 0, 2))


LAST_PROFILE = None


def kernel(trg, h0, embed_table, W_ih, W_hh, b_ih, b_hh, W_proj, b_proj):
    global LAST_PROFILE
    trg = np.asarray(trg)
    h0 = np.asarray(h0, dtype=np.float32)
    embed_table = np.asarray(embed_table, dtype=np.float32)
    W_ih = np.asarray(W_ih, dtype=np.float32)
    W_hh = np.asarray(W_hh, dtype=np.float32)
    b_ih = np.asarray(b_ih, dtype=np.float32)
    b_hh = np.asarray(b_hh, dtype=np.float32)
    W_proj = np.asarray(W_proj, dtype=np.float32)
    b_proj = np.asarray(b_proj, dtype=np.float32)

    # bx = b_ih + [b_hh for r,z chunks; 0 for n chunks], packed [128, GC]
    bx = b_ih.copy()
    bx[:2 * H] += b_hh[:2 * H]
    bhh_n_nonzero = bool(np.any(b_hh[2 * H:]))
    bproj_nonzero = bool(np.any(b_proj))
    bx_nonzero = bool(np.any(bx))
    nc = _build(bhh_n_nonzero, bproj_nonzero, bx_nonzero)

    # host-side layout prep (sharding/packing only)
    trg_flat = np.ascontiguousarray(
        trg[:, :S].T.reshape(NROW, 1)).astype(np.int32)
    tbl_bf = embed_table.astype(ml_dtypes.bfloat16)
    wih_t = _pack_T(W_ih, KE)
    whh_t = _pack_T(W_hh, KH)
    h0_t = np.ascontiguousarray(
        h0[0].T.reshape(KH, 128, B).transpose(1, 0, 2)).astype(ml_dtypes.bfloat16)
    bx_t = np.ascontiguousarray(bx.reshape(GC, 128).T).astype(ml_dtypes.bfloat16)

    base = {
        "trg_flat": trg_flat,
        "emb_tbl": tbl_bf,
        "wih_t": wih_t,
        "whh_t": whh_t,
        "h0_t": h0_t,
        "bx_t": bx_t,
    }
    if bhh_n_nonzero:
        base["bhn_t"] = np.ascontiguousarray(
            b_hh[2 * H:].reshape(KH, 128).T).astype(ml_dtypes.bfloat16)

    in_maps = []
    for c in range(NCORES):
        m = dict(base)
        m["wproj_t"] = _pack_T(W_proj[c * VS:(c + 1) * VS], KH,
                               dtype=ml_dtypes.float8_e4m3, scale=WSCL)
        if bproj_nonzero:
            m["bproj_s"] = np.ascontiguousarray(
                (b_proj[c * VS:(c + 1) * VS] * PSCL).reshape(1, VS))
        in_maps.append(m)

    trace = bool(int(os.environ.get("KERNEL_TRACE", "0")))
    res = run_bass_kernel_spmd(nc, in_maps, core_ids=list(range(NCORES)),
                               trace=trace)
    LAST_PROFILE = res

    out = np.zeros((B, T, V), dtype=np.float32)
    big = np.stack([res.results[c]["out_lp"].reshape(S, B, VS)
                    for c in range(NCORES)], axis=0)   # [c, t, b, vs]
    out[:, 1:, :] = big.transpose(2, 1, 0, 3).reshape(B, S, V)
    return out
